# revision 16
# baseline (speedup 1.0000x reference)
"""Trainium2 Bass kernel for nn_CLTBernoulliDecoder (CLT Bernoulli decoder loss).

Reference computation:
    logits = (z @ W + b).reshape(Bz, F, 2)        # interleaved states
    root fix: logits[:, root, 0] := logits[:, root, 1]
    xt = x[:, tree] ;  x_cond = stack([1-xt, xt])
    ls, lsn = log_sigmoid(+-logits)
    out[b,i] = sum_{j,s} x_cond*x * ls + x_cond*(1-x) * lsn

Algebraic restructuring used here (exact, not an approximation):
    log_sigmoid(t) = t - softplus(t)
    =>  out[b,i] = G[b,:]@z[i,:] + h[b]              (linear term, folded through W)
                 + sum_j U[b,j] * SP0[i,j]           (U = xt' - 1)
                 + sum_j V[b,j] * SP1[i,j]           (V = -xt')
    where SP_s = softplus(z @ W_s + b_s)  (W_s = W[:, s::2]),
          xt'[b,j] = 1 at roots else x[b, tree[j]],
          G = A_hat @ W.T,  h = A_hat @ b,
          A_hat[b, 2j+s] interleaves ((1-xt')*x, xt'*x).
    The root fix is exactly equivalent to setting xt' = 1 at root features.

softplus is evaluated as Ln(1 + Exp(l)) -- exp and ln share one ACT table set.
Biases ride along the matmuls as a 65th contraction row (z' has a ones row).

Sharding: data-parallel over Bz (4096 -> 8 x 512). x-derived coefficient
matrices are replicated; per-core outputs [256, 512] are concatenated on
axis 1 to form the full [256, 4096] result.
"""

import numpy as np
import ml_dtypes

BF16 = ml_dtypes.bfloat16

# Problem dimensions (hardcoded per spec).
BX = 256          # data points
BZ = 4096         # latent samples
ZD = 64           # latent dim
F = 784           # features
FP = 896          # features padded to 7*128
NT = FP // 128    # 7 j-tiles
N_CORES = 8
BZS = BZ // N_CORES  # 512 per core

_CACHE = {}


def _build_bass():
    import concourse.bass as bass
    import concourse.mybir as mybir
    import concourse.tile as tile
    from concourse import bacc
    from concourse.hw_specs import get_activation_tables

    fp32 = mybir.dt.float32
    bf16 = mybir.dt.bfloat16
    EXP = mybir.ActivationFunctionType.Exp
    LN = mybir.ActivationFunctionType.Ln

    class _Bacc(bacc.Bacc):
        """Pin Exp and Ln to the one table set holding both, so the table
        is loaded once instead of ping-ponging between per-function sets
        (~1.3us per reload)."""

        def insert_act_table_loads(self):
            has_activation = any(
                isinstance(i, mybir.InstActivation)
                for b in self.main_func.blocks
                for i in b.instructions
            )
            if not has_activation:
                return
            tables = []
            for name, funcs in get_activation_tables(self.m.arch).items():
                if name != "natural_log_exp_and_others":
                    funcs = {f for f in funcs if f not in (EXP, LN)}
                tables.append((name, funcs))
            import bass_rust as _bass_rust
            _bass_rust.insert_act_table_loads(self, tables)

    nc = _Bacc(None, target_bir_lowering=False)

    # wz = [w01 | z'] on 65 partitions; guv1 = [V | gp-padded] on 128
    d_wz = nc.dram_tensor("wz", [ZD + 1, 2 * FP + BZS], bf16, kind="ExternalInput")
    d_guv1 = nc.dram_tensor("guv1", [128, NT * BX + BX], bf16, kind="ExternalInput")
    d_uv0 = nc.dram_tensor("uv0", [128, NT, BX], bf16, kind="ExternalInput")
    d_out = nc.dram_tensor("out", [BX, BZS], fp32, kind="ExternalOutput")

    with tile.TileContext(nc) as tc:
        with (
            tc.tile_pool(name="singles", bufs=1) as singles,
            tc.tile_pool(name="outs", bufs=2) as outs_pool,
            tc.tile_pool(name="psum_l", bufs=1, space="PSUM") as psum_l,
            tc.tile_pool(name="psum_o", bufs=1, space="PSUM") as psum_o,
        ):
            # ---- PE warm-up: trip the HAM clock gate to 2.4 GHz while the
            # input DMAs land (needs sustained full-array activity) ----
            wu_sb = singles.tile([128, BZS], bf16)
            nc.gpsimd.memset(wu_sb, 0.0)
            wu_ps = psum_o.tile([128, BZS], fp32, tag="out0", name="wu_ps")
            for _ in range(4):
                nc.tensor.matmul(wu_ps, wu_sb[:, 0:128], wu_sb,
                                 start=True, stop=True)

            # ---- load inputs into SBUF (two HWDGE queues, merged
            # transfers so the critical weights+z arrive in one shot) ----
            wz = singles.tile([ZD + 1, 2 * FP + BZS], bf16)
            nc.sync.dma_start(out=wz, in_=d_wz[:])
            w01 = wz[:, 0:2 * FP].rearrange("p (s f) -> p s f", s=2)
            zp = wz[:, 2 * FP:]
            guv1 = singles.tile([128, NT * BX + BX], bf16)
            nc.scalar.dma_start(out=guv1, in_=d_guv1[:])
            v_sb = guv1[:, 0:NT * BX].rearrange("p (t b) -> p t b", t=NT)
            gp = guv1[0:ZD + 1, NT * BX:]
            u_sb3 = singles.tile([128, NT, BX], bf16)
            nc.sync.dma_start(out=u_sb3, in_=d_uv0[:])
            uv = [u_sb3, v_sb]

            # ---- persistent accumulators / staging ----
            # e/sp layout: [p, tile, state, i]
            out_ps = [psum_o.tile([128, BZS], fp32, tag=f"out{m}", name=f"out_ps{m}")
                      for m in range(2)]
            e_all = singles.tile([128, NT, 2, BZS], fp32)
            sp_all = singles.tile([128, NT, 2, BZS], bf16)
            e_flat = e_all.rearrange("p t s i -> p (t s i)")
            sp_flat = sp_all.rearrange("p t s i -> p (t s i)")

            def logits_mms(ta, tb, tag):
                # combined-state logits PSUM tile for tiles [ta, tb):
                # layout [p, (t, s), i]
                w = (tb - ta) * 2 * BZS
                l01 = psum_l.tile([128, w], fp32, tag=tag, name=f"l01_{ta}")
                for k, t in enumerate(range(ta, tb)):
                    for s in range(2):
                        ks = slice((2 * k + s) * BZS, (2 * k + s + 1) * BZS)
                        nc.tensor.matmul(l01[:, ks],
                                         w01[:, s, t * 128:(t + 1) * 128],
                                         zp, start=True, stop=True)
                return l01

            def exp_op(l01, ta, tb):
                nc.scalar.activation(
                    e_flat[:, ta * 2 * BZS:tb * 2 * BZS], l01, EXP)

            def ln_op(ta, tb):
                sl = slice(ta * 2 * BZS, tb * 2 * BZS)
                nc.scalar.activation(sp_flat[:, sl], e_flat[:, sl], LN, bias=1.0)

            def main_mms(ta, tb, last=False):
                for t in range(ta, tb):
                    for s in range(2):
                        for m in range(2):
                            fin = last and t == tb - 1 and s == 1 and m == 1
                            nc.tensor.matmul(
                                out_ps[m], uv[s][:, t, m * 128:(m + 1) * 128],
                                sp_all[:, t, s, :], start=False, stop=fin)

            # ---- schedule (ACT saturated; PE runs ahead via A/B PSUM) ----
            lB = logits_mms(0, 1, "lB")          # tile 0 (small first chunk
            exp_op(lB, 0, 1)                     #  so ACT starts early)
            # linear term opens the output accumulation group
            for m in range(2):
                nc.tensor.matmul(out_ps[m], gp[:, m * 128:(m + 1) * 128],
                                 zp, start=True, stop=False)
            lA = logits_mms(1, 3, "lA")          # tiles 1-2
            exp_op(lA, 1, 3)
            ln_op(0, 1)
            main_mms(0, 1)
            lA2 = logits_mms(3, 5, "lA")         # tiles 3-4
            exp_op(lA2, 3, 5)
            ln_op(1, 3)
            main_mms(1, 3)
            lB = logits_mms(5, 6, "lB")          # tile 5
            exp_op(lB, 5, 6)
            ln_op(3, 5)
            main_mms(3, 5)
            lB = logits_mms(6, 7, "lB")          # tile 6
            exp_op(lB, 6, 7)
            ln_op(5, 6)
            main_mms(5, 6)
            ln_op(6, 7)
            main_mms(6, 7, last=True)

            # ---- evict (ACT + DVE copies in parallel, two DMA queues) ----
            o0 = outs_pool.tile([128, BZS], fp32, tag="o0", name="o0")
            nc.scalar.copy(o0, out_ps[0])
            nc.sync.dma_start(out=d_out[0:128, :], in_=o0)
            o1 = outs_pool.tile([128, BZS], fp32, tag="o1", name="o1")
            nc.vector.tensor_copy(o1, out_ps[1])
            nc.scalar.dma_start(out=d_out[128:256, :], in_=o1)

    nc.compile()
    return nc


def _host_prep(x, z, W, b, tree):
    x = np.asarray(x, dtype=np.float32)
    z = np.asarray(z, dtype=np.float32)
    W = np.asarray(W, dtype=np.float32)
    b = np.asarray(b, dtype=np.float32)
    tree = np.asarray(tree, dtype=np.int64)

    root = tree < 0
    xt = x[:, tree]              # -1 wraps to last column, same as the ref
    xt[:, root] = 1.0            # root fix folded into coefficients

    # A_hat (interleaved): a0 = (1-xt')*x, a1 = xt'*x  (root rows give (0, x))
    Ahat = np.empty((BX, 2 * F), dtype=np.float32)
    Ahat[:, 0::2] = (1.0 - xt) * x
    Ahat[:, 1::2] = xt * x
    G = Ahat @ W.T               # [BX, ZD]
    h = Ahat @ b                 # [BX]

    # gp: [65, 256] = [G.T; h]
    gp = np.zeros((ZD + 1, BX), dtype=np.float32)
    gp[:ZD] = G.T
    gp[ZD] = h
    gp = gp.astype(BF16)

    # w01: [65, 2, 896] de-interleaved, bias as row 64, zero padded
    w01 = np.zeros((ZD + 1, 2, FP), dtype=np.float32)
    w01[:ZD, 0, :F] = W[:, 0::2]
    w01[:ZD, 1, :F] = W[:, 1::2]
    w01[ZD, 0, :F] = b[0::2]
    w01[ZD, 1, :F] = b[1::2]
    w01 = w01.astype(BF16)

    # uv0/uv1: [128, 7, 256]: U = xt'-1, V = -xt' (0 on padded features)
    U = np.zeros((FP, BX), dtype=np.float32)
    V = np.zeros((FP, BX), dtype=np.float32)
    U[:F] = xt.T - 1.0
    V[:F] = -xt.T
    uv0 = np.ascontiguousarray(U.reshape(NT, 128, BX).transpose(1, 0, 2)).astype(BF16)
    uv1 = np.ascontiguousarray(
        V.reshape(NT, 128, BX).transpose(1, 0, 2)).astype(BF16).reshape(128, NT * BX)

    # guv1 = [V | gp padded to 128 rows]: [128, 7*256 + 256]
    gp_pad = np.zeros((128, BX), dtype=BF16)
    gp_pad[:ZD + 1] = gp
    guv1 = np.ascontiguousarray(np.concatenate([uv1, gp_pad], axis=1))

    # z': [65, 4096] with ones row (bias channel)
    zp = np.ones((ZD + 1, BZ), dtype=np.float32)
    zp[:ZD] = z.T
    zp = zp.astype(BF16)

    # wz = [w01 flattened | z' shard]: [65, 2*896 + 512]
    w01_flat = w01.reshape(ZD + 1, 2 * FP)

    rep = {"guv1": guv1, "uv0": uv0}
    in_maps = []
    for c in range(N_CORES):
        m = dict(rep)
        m["wz"] = np.ascontiguousarray(
            np.concatenate([w01_flat, zp[:, c * BZS:(c + 1) * BZS]], axis=1))
        in_maps.append(m)
    return in_maps


def kernel(x, z, W, b, tree, **_unused):
    import os
    from concourse.bass_utils import run_bass_kernel_spmd

    if "nc" not in _CACHE:
        _CACHE["nc"] = _build_bass()
    nc = _CACHE["nc"]

    in_maps = _host_prep(x, z, W, b, tree)
    res = run_bass_kernel_spmd(nc, in_maps, core_ids=list(range(N_CORES)),
                               tmpdir=os.environ.get("BASS_TMPDIR"))
    _CACHE["last_result"] = res
    out = np.concatenate([res.results[c]["out"] for c in range(N_CORES)], axis=1)
    return out.astype(np.float32)


# revision 17
# speedup vs baseline: 1.0388x; 1.0388x over previous
"""Trainium2 Bass kernel for nn_CLTBernoulliDecoder (CLT Bernoulli decoder loss).

Reference computation:
    logits = (z @ W + b).reshape(Bz, F, 2)        # interleaved states
    root fix: logits[:, root, 0] := logits[:, root, 1]
    xt = x[:, tree] ;  x_cond = stack([1-xt, xt])
    ls, lsn = log_sigmoid(+-logits)
    out[b,i] = sum_{j,s} x_cond*x * ls + x_cond*(1-x) * lsn

Algebraic restructuring used here (exact, not an approximation):
    log_sigmoid(t) = t - softplus(t)
    =>  out[b,i] = G[b,:]@z[i,:] + h[b]              (linear term, folded through W)
                 + sum_j U[b,j] * SP0[i,j]           (U = xt' - 1)
                 + sum_j V[b,j] * SP1[i,j]           (V = -xt')
    where SP_s = softplus(z @ W_s + b_s)  (W_s = W[:, s::2]),
          xt'[b,j] = 1 at roots else x[b, tree[j]],
          G = A_hat @ W.T,  h = A_hat @ b,
          A_hat[b, 2j+s] interleaves ((1-xt')*x, xt'*x).
    The root fix is exactly equivalent to setting xt' = 1 at root features.

softplus is evaluated as Ln(1 + Exp(l)) -- exp and ln share one ACT table set.
Biases ride along the matmuls as a 65th contraction row (z' has a ones row).

Sharding: data-parallel over Bz (4096 -> 8 x 512). x-derived coefficient
matrices are replicated; per-core outputs [256, 512] are concatenated on
axis 1 to form the full [256, 4096] result.
"""

import numpy as np
import ml_dtypes

BF16 = ml_dtypes.bfloat16

# Problem dimensions (hardcoded per spec).
BX = 256          # data points
BZ = 4096         # latent samples
ZD = 64           # latent dim
F = 784           # features
FP = 896          # features padded to 7*128
NT = FP // 128    # 7 j-tiles
N_CORES = 8
BZS = BZ // N_CORES  # 512 per core

_CACHE = {}


def _build_bass():
    import concourse.bass as bass
    import concourse.mybir as mybir
    import concourse.tile as tile
    from concourse import bacc
    from concourse.hw_specs import get_activation_tables

    fp32 = mybir.dt.float32
    bf16 = mybir.dt.bfloat16
    EXP = mybir.ActivationFunctionType.Exp
    LN = mybir.ActivationFunctionType.Ln

    class _Bacc(bacc.Bacc):
        """Pin Exp and Ln to the one table set holding both, so the table
        is loaded once instead of ping-ponging between per-function sets
        (~1.3us per reload)."""

        def insert_act_table_loads(self):
            has_activation = any(
                isinstance(i, mybir.InstActivation)
                for b in self.main_func.blocks
                for i in b.instructions
            )
            if not has_activation:
                return
            tables = []
            for name, funcs in get_activation_tables(self.m.arch).items():
                if name != "natural_log_exp_and_others":
                    funcs = {f for f in funcs if f not in (EXP, LN)}
                tables.append((name, funcs))
            import bass_rust as _bass_rust
            _bass_rust.insert_act_table_loads(self, tables)

    nc = _Bacc(None, target_bir_lowering=False)

    d_w01 = nc.dram_tensor("w01", [ZD + 1, 2, FP], bf16, kind="ExternalInput")
    d_zp = nc.dram_tensor("zp", [ZD + 1, BZS], bf16, kind="ExternalInput")
    d_gp = nc.dram_tensor("gp", [ZD + 1, BX], bf16, kind="ExternalInput")
    d_uv0 = nc.dram_tensor("uv0", [128, NT, BX], bf16, kind="ExternalInput")
    d_uv1 = nc.dram_tensor("uv1", [128, NT, BX], bf16, kind="ExternalInput")
    d_out = nc.dram_tensor("out", [BX, BZS], fp32, kind="ExternalOutput")

    with tile.TileContext(nc) as tc:
        with (
            tc.tile_pool(name="singles", bufs=1) as singles,
            tc.tile_pool(name="outs", bufs=2) as outs_pool,
            tc.tile_pool(name="psum_l", bufs=1, space="PSUM") as psum_l,
            tc.tile_pool(name="psum_o", bufs=1, space="PSUM") as psum_o,
        ):
            # ---- PE warm-up: trip the HAM clock gate to 2.4 GHz while the
            # input DMAs land (needs sustained full-array activity) ----
            wu_sb = singles.tile([128, BZS], bf16)
            nc.gpsimd.memset(wu_sb, 0.0)
            wu_ps = psum_o.tile([128, BZS], fp32, tag="out0", name="wu_ps")
            for _ in range(8):
                nc.tensor.matmul(wu_ps, wu_sb[:, 0:128], wu_sb,
                                 start=True, stop=True)

            # ---- load inputs into SBUF (two HWDGE queues) ----
            zp = singles.tile([ZD + 1, BZS], bf16)
            nc.sync.dma_start(out=zp, in_=d_zp[:])
            w01 = singles.tile([ZD + 1, 2, FP], bf16)
            nc.sync.dma_start(out=w01, in_=d_w01[:])
            u_sb = singles.tile([128, NT, BX], bf16)
            nc.sync.dma_start(out=u_sb, in_=d_uv0[:])
            gp = singles.tile([ZD + 1, BX], bf16)
            nc.scalar.dma_start(out=gp, in_=d_gp[:])
            v_sb = singles.tile([128, NT, BX], bf16)
            nc.scalar.dma_start(out=v_sb, in_=d_uv1[:])
            uv = [u_sb, v_sb]

            # ---- persistent accumulators / staging ----
            # e/sp layout: [p, tile, state, i]
            out_ps = [psum_o.tile([128, BZS], fp32, tag=f"out{m}", name=f"out_ps{m}")
                      for m in range(2)]
            e_all = singles.tile([128, NT, 2, BZS], fp32)
            sp_all = singles.tile([128, NT, 2, BZS], bf16)
            e_flat = e_all.rearrange("p t s i -> p (t s i)")
            sp_flat = sp_all.rearrange("p t s i -> p (t s i)")

            def logits_mms(ta, tb, tag):
                # combined-state logits PSUM tile for tiles [ta, tb):
                # layout [p, (t, s), i]
                w = (tb - ta) * 2 * BZS
                l01 = psum_l.tile([128, w], fp32, tag=tag, name=f"l01_{ta}")
                for k, t in enumerate(range(ta, tb)):
                    for s in range(2):
                        ks = slice((2 * k + s) * BZS, (2 * k + s + 1) * BZS)
                        nc.tensor.matmul(l01[:, ks],
                                         w01[:, s, t * 128:(t + 1) * 128],
                                         zp, start=True, stop=True)
                return l01

            def exp_op(l01, ta, tb):
                nc.scalar.activation(
                    e_flat[:, ta * 2 * BZS:tb * 2 * BZS], l01, EXP)

            def ln_op(ta, tb):
                sl = slice(ta * 2 * BZS, tb * 2 * BZS)
                nc.scalar.activation(sp_flat[:, sl], e_flat[:, sl], LN, bias=1.0)

            def main_mms(ta, tb, last=False):
                for t in range(ta, tb):
                    for s in range(2):
                        for m in range(2):
                            fin = last and t == tb - 1 and s == 1 and m == 1
                            nc.tensor.matmul(
                                out_ps[m], uv[s][:, t, m * 128:(m + 1) * 128],
                                sp_all[:, t, s, :], start=False, stop=fin)

            # ---- schedule (ACT saturated; PE runs ahead via A/B PSUM) ----
            lB = logits_mms(0, 1, "lB")          # tile 0 (small first chunk
            exp_op(lB, 0, 1)                     #  so ACT starts early)
            # linear term opens the output accumulation group
            for m in range(2):
                nc.tensor.matmul(out_ps[m], gp[:, m * 128:(m + 1) * 128],
                                 zp, start=True, stop=False)
            lA = logits_mms(1, 3, "lA")          # tiles 1-2
            exp_op(lA, 1, 3)
            ln_op(0, 1)
            main_mms(0, 1)
            lA2 = logits_mms(3, 5, "lA")         # tiles 3-4
            exp_op(lA2, 3, 5)
            ln_op(1, 3)
            main_mms(1, 3)
            lB = logits_mms(5, 6, "lB")          # tile 5
            exp_op(lB, 5, 6)
            ln_op(3, 5)
            main_mms(3, 5)
            lB = logits_mms(6, 7, "lB")          # tile 6
            exp_op(lB, 6, 7)
            ln_op(5, 6)
            main_mms(5, 6)
            ln_op(6, 7)
            main_mms(6, 7, last=True)

            # ---- evict (ACT + DVE copies in parallel, two DMA queues) ----
            o0 = outs_pool.tile([128, BZS], fp32, tag="o0", name="o0")
            nc.scalar.copy(o0, out_ps[0])
            nc.sync.dma_start(out=d_out[0:128, :], in_=o0)
            o1 = outs_pool.tile([128, BZS], fp32, tag="o1", name="o1")
            nc.vector.tensor_copy(o1, out_ps[1])
            nc.scalar.dma_start(out=d_out[128:256, :], in_=o1)

    nc.compile()
    return nc


def _host_prep(x, z, W, b, tree):
    x = np.asarray(x, dtype=np.float32)
    z = np.asarray(z, dtype=np.float32)
    W = np.asarray(W, dtype=np.float32)
    b = np.asarray(b, dtype=np.float32)
    tree = np.asarray(tree, dtype=np.int64)

    root = tree < 0
    xt = x[:, tree]              # -1 wraps to last column, same as the ref
    xt[:, root] = 1.0            # root fix folded into coefficients

    # A_hat (interleaved): a0 = (1-xt')*x, a1 = xt'*x  (root rows give (0, x))
    Ahat = np.empty((BX, 2 * F), dtype=np.float32)
    Ahat[:, 0::2] = (1.0 - xt) * x
    Ahat[:, 1::2] = xt * x
    G = Ahat @ W.T               # [BX, ZD]
    h = Ahat @ b                 # [BX]

    # gp: [65, 256] = [G.T; h]
    gp = np.zeros((ZD + 1, BX), dtype=np.float32)
    gp[:ZD] = G.T
    gp[ZD] = h
    gp = gp.astype(BF16)

    # w01: [65, 2, 896] de-interleaved, bias as row 64, zero padded
    w01 = np.zeros((ZD + 1, 2, FP), dtype=np.float32)
    w01[:ZD, 0, :F] = W[:, 0::2]
    w01[:ZD, 1, :F] = W[:, 1::2]
    w01[ZD, 0, :F] = b[0::2]
    w01[ZD, 1, :F] = b[1::2]
    w01 = w01.astype(BF16)

    # uv0/uv1: [128, 7, 256]: U = xt'-1, V = -xt' (0 on padded features)
    U = np.zeros((FP, BX), dtype=np.float32)
    V = np.zeros((FP, BX), dtype=np.float32)
    U[:F] = xt.T - 1.0
    V[:F] = -xt.T
    uv0 = np.ascontiguousarray(U.reshape(NT, 128, BX).transpose(1, 0, 2)).astype(BF16)
    uv1 = np.ascontiguousarray(V.reshape(NT, 128, BX).transpose(1, 0, 2)).astype(BF16)

    # z': [65, 4096] with ones row (bias channel)
    zp = np.ones((ZD + 1, BZ), dtype=np.float32)
    zp[:ZD] = z.T
    zp = zp.astype(BF16)

    rep = {"w01": w01, "gp": gp, "uv0": uv0, "uv1": uv1}
    in_maps = []
    for c in range(N_CORES):
        m = dict(rep)
        m["zp"] = np.ascontiguousarray(zp[:, c * BZS:(c + 1) * BZS])
        in_maps.append(m)
    return in_maps


def kernel(x, z, W, b, tree, **_unused):
    import os
    from concourse.bass_utils import run_bass_kernel_spmd

    if "nc" not in _CACHE:
        _CACHE["nc"] = _build_bass()
    nc = _CACHE["nc"]

    in_maps = _host_prep(x, z, W, b, tree)
    res = run_bass_kernel_spmd(nc, in_maps, core_ids=list(range(N_CORES)),
                               tmpdir=os.environ.get("BASS_TMPDIR"))
    _CACHE["last_result"] = res
    out = np.concatenate([res.results[c]["out"] for c in range(N_CORES)], axis=1)
    return out.astype(np.float32)
